# revision 1
# baseline (speedup 1.0000x reference)
# Linformer attention kernel for Trainium2 (8 NeuronCores, SPMD, no collectives).
#
# Sharding (matches the hint): core c = (batch b = c//2, head-group hg = c%2).
# Each core runs one batch's full Linformer attention for 8 of the 16 heads:
#   Wq/Wk/Wv column-sliced (512 cols), Wo row-sliced (512 rows) -> partial
#   [L, D] output; the host sums the two partials per batch.
#
# Device kernel (per core), all matmuls in float32r (full PE rate):
#   P1: qT = (Wq*SCALE)^T-side projection from host-pretransposed xT;
#       kk/vv projected and immediately reduced over L into k_projT / v_proj
#       PSUM accumulators (Linformer low-rank projection).
#   P2: per head, transposed logits (K on partitions, L on free) so no Q or
#       attn transposes are needed; exp on ScalarE; attn@v with a ones column
#       appended to v_proj so the softmax denominator lands in the same PSUM
#       tile; normalization fused into the PSUM evacuation (per-partition
#       scalar multiply). Softmax skips max-subtraction: |logits*scale| < ~6
#       for this problem, exp is safely in range.
#   P3: per L-tile, PE-transpose of the head outputs + Wo matmul -> y.

import os
import sys

for _p in ("/opt/trn_rl_repo", "/root/.axon_site/_ro/trn_rl_repo"):
    if os.path.isdir(_p) and _p not in sys.path:
        sys.path.insert(0, _p)
        break

import numpy as np

import concourse.bass as bass
from concourse import bacc
import concourse.mybir as mybir
from concourse.bass_utils import run_bass_kernel_spmd
from concourse.tile import TileContext

P = 128
B, L, D = 4, 4096, 1024
H, HD = 16, 64
K = 256
SCALE = HD ** -0.5

NCORES = 8
HG = 2                 # head groups (cores per batch)
DG = D // HG           # 512: local width of Wq/Wk/Wv cols & Wo rows
HL = H // HG           # 8 local heads
KC = D // P            # 8 contraction chunks over D
DGT = DG // P          # 4 partition-tiles over local d
KT = K // P            # 2 partition-tiles over low-rank K
LB = 512               # L block for phase 1/2
NLB = L // LB          # 8
NLT = L // P           # 32 L-tiles
VW = 68                # padded per-head v_proj row width (64 + ones col @64)

f32 = mybir.dt.float32
f32r = mybir.dt.float32r
AF = mybir.ActivationFunctionType


def build_kernel(nc: bass.Bass):
    xT = nc.dram_tensor("xT", (D, L), f32r, kind="ExternalInput")
    wq = nc.dram_tensor("wq", (D, DG), f32r, kind="ExternalInput")
    wk = nc.dram_tensor("wk", (D, DG), f32r, kind="ExternalInput")
    wv = nc.dram_tensor("wv", (D, DG), f32r, kind="ExternalInput")
    wo = nc.dram_tensor("wo", (DG, D), f32r, kind="ExternalInput")
    pk = nc.dram_tensor("pk", (L, K), f32r, kind="ExternalInput")
    pv = nc.dram_tensor("pv", (L, K), f32r, kind="ExternalInput")
    idin = nc.dram_tensor("idin", (P, P), f32, kind="ExternalInput")
    y = nc.dram_tensor("y", (L, D), f32, kind="ExternalOutput")

    xT_r = xT.rearrange("(kc p) l -> kc p l", p=P)      # [8,128,L]
    wq_r = wq.rearrange("(kc p) m -> kc p m", p=P)      # [8,128,512]
    wk_r = wk.rearrange("(kc p) m -> kc p m", p=P)
    wv_r = wv.rearrange("(kc p) m -> kc p m", p=P)
    wo_r = wo.rearrange("(dt p) n -> dt p n", p=P)      # [4,128,1024]
    pk_r = pk.rearrange("(lt p) k -> lt p k", p=P)      # [32,128,256]
    pv_r = pv.rearrange("(lt p) k -> lt p k", p=P)
    y_r = y.rearrange("(lt p) n -> lt p n", p=P)        # [32,128,1024]

    with TileContext(nc) as tc:
        with tc.tile_pool(name="const", bufs=1) as cpool:
            # Resident tensors (per-partition bytes in comments).
            wq_sb = cpool.tile([P, KC, DG], f32r)        # 16K
            wk_sb = cpool.tile([P, KC, DG], f32r)        # 16K
            wv_sb = cpool.tile([P, KC, DG], f32r)        # 16K
            wo_sb = cpool.tile([P, DGT, D], f32r)        # 16K
            qT_sb = cpool.tile([P, DGT, L], f32r)        # 64K
            kpT_sb = cpool.tile([P, DGT, K], f32r)       # 4K   k_proj^T [d, K]
            vpa_sb = cpool.tile([P, KT, HL, VW], f32r)   # 4.25K v_proj + ones col
            ident = cpool.tile([P, P], f32)             # 0.5K

            nc.sync.dma_start(ident[:], idin[:, :])
            nc.vector.memset(vpa_sb[:].bitcast(f32), 1.0)            # bakes the ones column

            nc.sync.dma_start(wq_sb[:], wq.rearrange("(kc p) m -> p kc m", p=P))
            nc.sync.dma_start(wk_sb[:], wk.rearrange("(kc p) m -> p kc m", p=P))
            nc.sync.dma_start(wv_sb[:], wv.rearrange("(kc p) m -> p kc m", p=P))
            nc.sync.dma_start(wo_sb[:], wo.rearrange("(dt p) n -> p dt n", p=P))

            # ---------------- Phase 1: projections + low-rank reduce --------
            with tc.tile_pool(name="p1_acc", bufs=1, space="PSUM") as accp, \
                 tc.tile_pool(name="p1_mm", bufs=2, space="PSUM") as mmp, \
                 tc.tile_pool(name="p1_x", bufs=2) as xsp, \
                 tc.tile_pool(name="p1_p", bufs=2) as psp, \
                 tc.tile_pool(name="p1_kv", bufs=3) as kvp:
                # one full PSUM bank per accumulation group (start=True clears
                # has_written for the whole bank -> groups must not share one)
                kpT_ps = accp.tile([P, DGT, 512], f32)  # 4 banks (256 used)
                vp_ps = accp.tile([P, KT, DG], f32)     # 2 banks

                for lb in range(NLB):
                    sl = slice(lb * LB, (lb + 1) * LB)
                    xTb = xsp.tile([P, KC, LB], f32r, tag="xTb")
                    xview = xT.rearrange("(kc p) l -> p kc l", p=P)
                    for kc2 in range(0, KC, 2):
                        nc.sync.dma_start(
                            xTb[:, kc2:kc2 + 2], xview[:, kc2:kc2 + 2, sl])

                    # qT[dpt, lb-block] = (Wq columns)^T @ x^T-block
                    for dt_ in range(DGT):
                        q_ps = mmp.tile([P, LB], f32, tag="mm")
                        for kc in range(KC):
                            nc.tensor.matmul(
                                q_ps,
                                (wq_sb[:, kc, dt_ * P:(dt_ + 1) * P]),
                                (xTb[:, kc]),
                                start=(kc == 0), stop=(kc == KC - 1),
                            )
                        nc.scalar.copy(qT_sb[:, dt_, sl], q_ps)

                    for j in range(LB // P):
                        lt = lb * (LB // P) + j
                        first, last = (lt == 0), (lt == NLT - 1)
                        jsl = slice(j * P, (j + 1) * P)

                        pkt = psp.tile([P, K], f32r, tag="pkt")
                        nc.sync.dma_start(pkt, pk_r[lt])
                        pvt = psp.tile([P, K], f32r, tag="pvt")
                        nc.sync.dma_start(pvt, pv_r[lt])

                        # kk tile [128 L, 512 d], then k_projT += kk^T-chunks @ pk
                        kk_ps = mmp.tile([P, DG], f32, tag="mm")
                        for kc in range(KC):
                            nc.tensor.matmul(
                                kk_ps, (xTb[:, kc, jsl]), (wk_sb[:, kc]),
                                start=(kc == 0), stop=(kc == KC - 1),
                            )
                        kk_sb = kvp.tile([P, DG], f32r, tag="kv")
                        nc.vector.tensor_copy(kk_sb, kk_ps)
                        for dt_ in range(DGT):
                            nc.tensor.matmul(
                                kpT_ps[:, dt_, :K],
                                (kk_sb[:, dt_ * P:(dt_ + 1) * P]), (pkt),
                                start=first, stop=last,
                            )

                        # vv tile, then v_proj += pv^T-chunks @ vv
                        vv_ps = mmp.tile([P, DG], f32, tag="mm")
                        for kc in range(KC):
                            nc.tensor.matmul(
                                vv_ps, (xTb[:, kc, jsl]), (wv_sb[:, kc]),
                                start=(kc == 0), stop=(kc == KC - 1),
                            )
                        vv_sb = kvp.tile([P, DG], f32r, tag="kv")
                        nc.vector.tensor_copy(vv_sb, vv_ps)
                        for kpt in range(KT):
                            nc.tensor.matmul(
                                vp_ps[:, kpt],
                                (pvt[:, kpt * P:(kpt + 1) * P]), (vv_sb),
                                start=first, stop=last,
                            )

                for dt_ in range(DGT):
                    nc.vector.tensor_copy(kpT_sb[:, dt_], kpT_ps[:, dt_, :K])
                for kpt in range(KT):
                    # strided copy into the padded per-head layout (ones col kept)
                    nc.vector.tensor_copy(
                        vpa_sb[:, kpt, :, 0:HD],
                        vp_ps[:, kpt].rearrange("p (h d) -> p h d", d=HD),
                    )

            # ---------------- Phase 2+3: attention + output projection ------
            with tc.tile_pool(name="p2_lg", bufs=2, space="PSUM") as lgp, \
                 tc.tile_pool(name="p2_av", bufs=2, space="PSUM") as avp, \
                 tc.tile_pool(name="p2_tr", bufs=2, space="PSUM") as trp, \
                 tc.tile_pool(name="p2_y", bufs=2, space="PSUM") as uyp, \
                 tc.tile_pool(name="p2_s", bufs=3) as sp2, \
                 tc.tile_pool(name="p2_e", bufs=4) as ep2, \
                 tc.tile_pool(name="p2_o", bufs=2) as op2, \
                 tc.tile_pool(name="p2_r", bufs=8) as rp2:
                for lb in range(NLB):
                    sl = slice(lb * LB, (lb + 1) * LB)
                    out_blk = op2.tile([P, LB // P, DG], f32, tag="oblk")
                    for h in range(HL):
                        dt_ = h // 2
                        off = (h % 2) * HD
                        hsl = slice(off, off + HD)
                        eT = []
                        for kpt in range(KT):
                            lg_ps = lgp.tile([P, LB], f32, tag="lg")
                            nc.tensor.matmul(
                                lg_ps,
                                (kpT_sb[hsl, dt_, kpt * P:(kpt + 1) * P]),
                                (qT_sb[hsl, dt_, sl]),
                            )
                            e_sb = ep2.tile([P, LB], f32r, tag="eT")
                            nc.scalar.activation(e_sb, lg_ps, AF.Exp)
                            eT.append(e_sb)
                        for j in range(LB // P):
                            jsl = slice(j * P, (j + 1) * P)
                            av_ps = avp.tile([P, VW], f32, tag="av")
                            for kpt in range(KT):
                                nc.tensor.matmul(
                                    av_ps[:],
                                    (eT[kpt][:, jsl]),
                                    (vpa_sb[:, kpt, h]),
                                    start=(kpt == 0), stop=(kpt == KT - 1),
                                )
                            r_sb = rp2.tile([P, 1], f32, tag="r")
                            nc.vector.reciprocal(r_sb, av_ps[:, HD:HD + 1])
                            nc.vector.tensor_scalar_mul(
                                out_blk[:, j, h * HD:(h + 1) * HD],
                                av_ps[:, 0:HD], r_sb,
                            )
                    # P3: transpose out rows, then Wo
                    for j in range(LB // P):
                        lt = lb * (LB // P) + j
                        oT_sb = sp2.tile([P, DGT, P], f32r, tag="oT")
                        for dt_ in range(DGT):
                            t_ps = trp.tile([P, P], f32, tag="tr")
                            nc.tensor.transpose(
                                t_ps, out_blk[:, j, dt_ * P:(dt_ + 1) * P], ident)
                            nc.vector.tensor_copy(oT_sb[:, dt_], t_ps)
                        y_sb = sp2.tile([P, D], f32, tag="ysb")
                        for n in range(2):
                            y_ps = uyp.tile([P, 512], f32, tag="yps")
                            for dt_ in range(DGT):
                                nc.tensor.matmul(
                                    y_ps,
                                    (oT_sb[:, dt_]),
                                    (wo_sb[:, dt_, n * 512:(n + 1) * 512]),
                                    start=(dt_ == 0), stop=(dt_ == DGT - 1),
                                )
                            nc.vector.tensor_copy(y_sb[:, n * 512:(n + 1) * 512], y_ps)
                        nc.sync.dma_start(y_r[lt], y_sb)
    return nc


_NC_CACHE = {}


def _get_nc():
    if "nc" not in _NC_CACHE:
        nc = bacc.Bacc("TRN2", debug=False, num_devices=NCORES)
        build_kernel(nc)
        nc.finalize()  # runs Bacc.compile(): wait splitting + reg alloc
        _NC_CACHE["nc"] = nc
    return _NC_CACHE["nc"]


def make_in_maps(x, Wq, Wk, Wv, Wo, proj_k, proj_v):
    f = np.float32
    x = np.asarray(x, f)
    Wq = np.asarray(Wq, f)
    Wk = np.asarray(Wk, f)
    Wv = np.asarray(Wv, f)
    Wo = np.asarray(Wo, f)
    pk = np.ascontiguousarray(np.asarray(proj_k, f))
    pv = np.ascontiguousarray(np.asarray(proj_v, f))
    in_maps = []
    for c in range(NCORES):
        b, hg = divmod(c, HG)
        cs = slice(hg * DG, (hg + 1) * DG)
        in_maps.append({
            "xT": np.ascontiguousarray(x[b].T),
            "wq": np.ascontiguousarray(Wq[:, cs] * SCALE),
            "wk": np.ascontiguousarray(Wk[:, cs]),
            "wv": np.ascontiguousarray(Wv[:, cs]),
            "wo": np.ascontiguousarray(Wo[cs, :]),
            "pk": pk,
            "pv": pv,
            "idin": np.eye(P, dtype=f),
        })
    return in_maps


def gather_output(results):
    outs = [results[c]["y"] for c in range(NCORES)]
    y = np.stack([outs[HG * b] + outs[HG * b + 1] for b in range(B)])
    return np.asarray(y, np.float32)


def kernel(x, Wq, Wk, Wv, Wo, proj_k, proj_v, _trace=False, _trace_kwargs=None):
    nc = _get_nc()
    in_maps = make_in_maps(x, Wq, Wk, Wv, Wo, proj_k, proj_v)
    res = run_bass_kernel_spmd(
        nc, in_maps, core_ids=list(range(NCORES)),
        trace=_trace, **(_trace_kwargs or {}),
    )
    out = gather_output(res.results)
    if _trace:
        return out, res
    return out

